# revision 1
# baseline (speedup 1.0000x reference)
"""Trainium2 Bass kernel for nn_Attention (B=2, N=2048, C=1024, H=16).

Sharding: tensor-parallel over heads — 2 heads per core on 8 cores.
Each core computes qkv/attention/proj-partial for its 2 heads over both
batches; the host sums the 8 proj partials and adds the bias.

Per-core layout choices (all matmul operands fp16, PSUM accumulation fp32):
  - host supplies x pre-transposed (xT [C, B*N]) so the embed contraction
    dim lands on SBUF partitions with plain contiguous DMAs
  - qT/kT computed as [128=(h0|h1 dims), tokens]; per-head slices sit at
    partition offsets 0/64 (matmul tile_position handles K=64 operands)
  - scores are computed transposed, ST = K^T-chunk @ Q^T -> [keys, queries],
    and softmax skips the max-subtraction (|scores*scale| < ~2.1 for this
    problem's data, far from fp32 exp range limits)
  - the softmax denominator comes free from the AV matmul via a ones column
    packed between the two heads' V columns ([v0 | 1 | v1]); outU col 64
    (h0) / col 0 (h1) is sum(exp(s)), normalized with a per-partition
    reciprocal multiply
  - attn output tiles are transposed on the tensor engine (128x128, fp16)
    so the proj matmul contracts both heads in a single K=128 shot
"""

import numpy as np
from contextlib import ExitStack

import concourse.bass as bass
import concourse.mybir as mybir
import concourse.tile as tile
from concourse import bacc
from concourse.bass import ts
from concourse.bass_utils import run_bass_kernel_spmd
from concourse.masks import make_identity

P = 128
B = 2
N = 2048
C = 1024
H = 16
D = 64
T = B * N            # 4096 tokens
KO = C // P          # 8 contraction chunks of 128
NCORES = 8
HPC = H // NCORES    # 2 heads per core
TB = 512             # token block for qkv / query block for attention
SCALE = C ** -0.5    # 1/32 — note: reference scales by embed_dim**-0.5

F16 = mybir.dt.float16
F32 = mybir.dt.float32


def build_program(n_iters: int = 1, hw_loop: int = 1):
    nc = bacc.Bacc("TRN2", target_bir_lowering=False, debug=False)

    xT = nc.dram_tensor("xT", [C, T], F16, kind="ExternalInput")
    wqkv = nc.dram_tensor("wqkv", [C, 3 * P], F16, kind="ExternalInput")
    wproj = nc.dram_tensor("wproj", [P, C], F16, kind="ExternalInput")
    y = nc.dram_tensor("y", [T, C], F16, kind="ExternalOutput")

    xT_r = xT.rearrange("(o p) t -> p o t", p=P)
    wqkv_r = wqkv.rearrange("(o p) c -> p o c", p=P)

    with tile.TileContext(nc) as tc, ExitStack() as ctx:
        QW = 1024  # exp/score tile width (queries)
        const = ctx.enter_context(tc.tile_pool(name="const", bufs=1))
        big = ctx.enter_context(tc.tile_pool(name="big", bufs=1))
        etp = ctx.enter_context(tc.tile_pool(name="etp", bufs=8))
        oup = ctx.enter_context(tc.tile_pool(name="oup", bufs=2))
        yp = ctx.enter_context(tc.tile_pool(name="yp", bufs=4))
        smalls = ctx.enter_context(tc.tile_pool(name="smalls", bufs=4))
        mmp = ctx.enter_context(tc.tile_pool(name="mmp", bufs=2, space="PSUM"))
        stp = ctx.enter_context(tc.tile_pool(name="stp", bufs=2, space="PSUM"))
        outup = ctx.enter_context(tc.tile_pool(name="outup", bufs=2, space="PSUM"))

        ident = const.tile([P, P], F16)
        make_identity(nc, ident)
        ident32 = const.tile([P, P], F32)
        make_identity(nc, ident32)
        wqkv_sb = const.tile([P, KO, 3 * P], F16)
        nc.sync.dma_start(wqkv_sb[:], wqkv_r)
        wproj_sb = const.tile([P, C], F16)
        nc.sync.dma_start(wproj_sb[:], wproj[:])

        def body():
            xT_sb = big.tile([P, KO, T], F16, tag="xT")
            for t in range(T // TB):
                nc.sync.dma_start(xT_sb[:, :, ts(t, TB)], xT_r[:, :, ts(t, TB)])

            qT_sb = big.tile([P, T], F16, tag="qT")
            kT_sb = big.tile([P, T], F16, tag="kT")
            # [v_h0 (64) | ones (1) | v_h1 (64)] per token chunk
            v_sb = big.tile([P, T // P, 129], F16, tag="v")
            aout_sb = big.tile([P, T // P, P], F16, tag="aout")
            aoutT_sb = big.tile([P, T // P, P], F16, tag="aoutT")
            nc.vector.memset(v_sb[:, :, 64:65], 1.0)

            # emission helpers — each emits one PE "work packet"
            def emit_qk_tile(m, dst, t):
                ps = mmp.tile([P, TB], F32, tag="mm", name="ps_qk")
                for k in range(KO):
                    nc.tensor.matmul(
                        ps[:],
                        lhsT=wqkv_sb[:, k, ts(m, P)],
                        rhs=xT_sb[:, k, ts(t, TB)],
                        start=(k == 0),
                        stop=(k == KO - 1),
                    )
                nc.vector.tensor_copy(dst[:, ts(t, TB)], ps[:])

            def emit_v_tile(t):
                ps = mmp.tile([P, TB], F32, tag="mm", name="ps_v")
                for k in range(KO):
                    nc.tensor.matmul(
                        ps[:, :P],
                        lhsT=xT_sb[:, k, ts(t, P)],
                        rhs=wqkv_sb[:, k, 2 * P : 3 * P],
                        start=(k == 0),
                        stop=(k == KO - 1),
                    )
                nc.vector.tensor_copy(v_sb[:, t, 0:64], ps[:, 0:64])
                nc.vector.tensor_copy(v_sb[:, t, 65:129], ps[:, 64:128])

            def emit_proj_chunk(t):
                # transpose [tok, hd] -> [hd, tok], then y = aoutT.T @ wproj
                pst = mmp.tile([P, P], F16, tag="mm", name="ps_tr")
                nc.tensor.transpose(pst[:], aout_sb[:, t, :], ident[:])
                nc.vector.tensor_copy(aoutT_sb[:, t, :], pst[:])
                for nb in range(C // TB):
                    ps = mmp.tile([P, TB], F32, tag="mm", name="ps_pr")
                    nc.tensor.matmul(
                        ps[:],
                        lhsT=aoutT_sb[:, t, :],
                        rhs=wproj_sb[:, ts(nb, TB)],
                        start=True,
                        stop=True,
                    )
                    yt = yp.tile([P, TB], F16, tag="y")
                    nc.vector.tensor_copy(yt[:], ps[:])
                    nc.sync.dma_start(y[ts(t, P), ts(nb, TB)], yt[:])

            from collections import deque
            fillers = deque()  # (key, fn) — emission order defines dep order
            emitted = set()

            def pop_filler():
                while fillers:
                    key, fn = fillers.popleft()
                    if key in emitted:
                        continue
                    emitted.add(key)
                    fn()
                    return

            def ensure_filler(key):
                if key in emitted:
                    return
                for k2, fn in fillers:
                    if k2 == key:
                        emitted.add(key)
                        fn()
                        return

            # ---- minimal QKV lead: kT(b0) + qT(b0 qb0); everything else
            # ---- (incl V) drains as filler during attention windows
            emit_qk_tile(1, kT_sb, 0)
            for t in range(2):
                emit_qk_tile(0, qT_sb, t)
            for t in range(1, 4):
                fillers.append((("qk", 1, t), lambda t=t: emit_qk_tile(1, kT_sb, t)))
            for t in range(2, 4):
                fillers.append((("qk", 0, t), lambda t=t: emit_qk_tile(0, qT_sb, t)))
            for t in range(16):
                fillers.append((("v", t), lambda t=t: emit_v_tile(t)))
            for t in range(4, 8):
                fillers.append((("qk", 1, t), lambda t=t: emit_qk_tile(1, kT_sb, t)))
            for t in range(4, 8):
                fillers.append((("qk", 0, t), lambda t=t: emit_qk_tile(0, qT_sb, t)))
            for t in range(16, 32):
                fillers.append((("v", t), lambda t=t: emit_v_tile(t)))

            # ---- attention: per batch, qb outer / head inner so proj chunks
            # ---- for (b, qb) become fillers for the next window
            for b in range(B):
                for qb in range(N // QW):
                    # guarantee this window's qT tiles are emitted first
                    # (kT tiles are ensure-pulled per kc, at first use)
                    for t in range(4 * b + 2 * qb, 4 * b + 2 * qb + 2):
                        ensure_filler(("qk", 0, t))
                    for h in range(HPC):
                        hs = h * 64
                        qTh = qT_sb[hs : hs + 64, b * N : (b + 1) * N]
                        kTh = kT_sb[hs : hs + 64, b * N : (b + 1) * N]
                        # ones col first for h1, last for h0
                        u_lo = 0 if h == 0 else 64
                        dcol = 64 if h == 0 else 0
                        o0 = 0 if h == 0 else 1
                        # swapped AV: U=[v|1] is the stationary operand (65-row
                        # ldweights hidden under 512-row ET streams); output is
                        # ouT [65, queries], accumulated per kc right behind
                        # each exp (1-deep software pipeline on the et tiles)
                        ouTs = [
                            outup.tile([P, TB], F32, tag="outu", name=f"ouT{i}")
                            for i in range(QW // TB)
                        ]

                        def emit_av(kc, et):
                            ensure_filler(("v", b * (N // P) + kc))
                            for half in range(QW // TB):
                                nc.tensor.matmul(
                                    ouTs[half][:65, :],
                                    lhsT=v_sb[:, b * (N // P) + kc,
                                              u_lo : u_lo + 65],
                                    rhs=et[:, ts(half, TB)],
                                    start=(kc == 0),
                                    stop=(kc == N // P - 1),
                                )

                        prev = None
                        for kc in range(N // P):
                            ensure_filler(("qk", 1, 4 * b + kc // 4))
                            st = stp.tile([P, QW], F32, tag="st")
                            for half in range(QW // TB):
                                nc.tensor.matmul(
                                    st[:, ts(half, TB)],
                                    lhsT=kTh[:, ts(kc, P)],
                                    rhs=qTh[:, qb * QW + half * TB :
                                            qb * QW + (half + 1) * TB],
                                    start=True,
                                    stop=True,
                                )
                            et = etp.tile([P, QW], F16, tag="et", name=f"et{kc}")
                            nc.scalar.activation(
                                et[:], st[:], mybir.ActivationFunctionType.Exp,
                                scale=SCALE,
                            )
                            if prev is not None:
                                emit_av(kc - 1, prev)
                            prev = et
                            pop_filler()
                            pop_filler()
                        emit_av(N // P - 1, prev)

                        # stage ouT to SBUF (fp32), transpose back to
                        # [queries, 65], then per-partition normalize
                        ou32 = oup.tile([P, QW], F32, tag="ou32")
                        for half in range(QW // TB):
                            nc.vector.tensor_copy(
                                ou32[:65, ts(half, TB)], ouTs[half][:65, :])
                        for qs in range(QW // P):
                            ptr = mmp.tile([P, P], F32, tag="mm", name="ps_ut")
                            nc.tensor.transpose(
                                ptr[:, :65], ou32[:65, ts(qs, P)], ident32[:65, :65])
                            rec = smalls.tile([P, 1], F32, tag="rec")
                            nc.vector.reciprocal(rec[:], ptr[:, dcol : dcol + 1])
                            tc_idx = b * (N // P) + qb * (QW // P) + qs
                            nc.vector.tensor_scalar_mul(
                                aout_sb[:, tc_idx, hs : hs + 64],
                                ptr[:, o0 : o0 + 64],
                                rec[:],
                            )
                            # last window: no more exp work exists for ACT, so
                            # emit proj right behind each normalize instead of
                            # leaving it as a serial tail after the loop
                            last_win = (b == B - 1 and qb == N // QW - 1
                                        and h == HPC - 1)
                            if last_win:
                                emit_proj_chunk(tc_idx)
                            else:
                                pop_filler()
                    # proj for these tokens becomes filler work
                    if not (b == B - 1 and qb == N // QW - 1):
                        for qs in range(QW // P):
                            t = b * (N // P) + qb * (QW // P) + qs
                            fillers.append(
                                (("proj", t), lambda t=t: emit_proj_chunk(t)))

            while fillers:
                pop_filler()

        if hw_loop > 1:
            with tc.For_i(0, hw_loop, 1):
                body()
        else:
            for _ in range(n_iters):
                body()

    nc.compile()
    return nc


_CACHE = {}


def _get_program(n_iters: int = 1):
    if n_iters not in _CACHE:
        _CACHE[n_iters] = build_program(n_iters)
    return _CACHE[n_iters]


def make_core_inputs(x, W_qkv):
    """Shared per-core host prep; returns (xT16, [wqkv_c for c in range(8)])."""
    xT16 = np.ascontiguousarray(
        x.reshape(T, C).astype(np.float16, copy=False).T
    )
    wq = []
    for c in range(NCORES):
        lo, hi = 2 * c * 64, (2 * c + 2) * 64
        wq.append(
            np.ascontiguousarray(
                np.concatenate(
                    [W_qkv[:, lo:hi], W_qkv[:, C + lo : C + hi],
                     W_qkv[:, 2 * C + lo : 2 * C + hi]],
                    axis=1,
                ).astype(np.float16)
            )
        )
    return xT16, wq


def kernel(x, W_qkv, W_proj, b_proj):
    x = np.asarray(x, dtype=np.float32)
    W_qkv = np.asarray(W_qkv, dtype=np.float32)
    W_proj = np.asarray(W_proj, dtype=np.float32)
    b_proj = np.asarray(b_proj, dtype=np.float32)

    nc = _get_program(1)
    xT16, wq = make_core_inputs(x, W_qkv)
    in_maps = []
    for c in range(NCORES):
        lo, hi = 2 * c * 64, (2 * c + 2) * 64
        in_maps.append(
            {
                "xT": xT16,
                "wqkv": wq[c],
                "wproj": np.ascontiguousarray(W_proj[lo:hi, :].astype(np.float16)),
            }
        )

    res = run_bass_kernel_spmd(nc, in_maps, list(range(NCORES)))
    acc = np.zeros((T, C), dtype=np.float32)
    for c in range(NCORES):
        acc += res.results[c]["y"].astype(np.float32)
    acc += b_proj[None, :]
    return acc.reshape(B, N, C)



# revision 5
# speedup vs baseline: 261.3691x; 261.3691x over previous
"""Trainium2 Bass kernel for nn_Attention (B=2, N=2048, C=1024, H=16).

Sharding: tensor-parallel over heads - 2 heads per core on 8 cores.
Each core computes qkv/attention/proj-partial for its 2 heads over both
batches; the host sums the 8 proj partials and adds the bias.

v2: quadrant-packed attention (ACT-bound design).
  - h0 lives on SBUF/PSUM partitions 0-63, h1 on 64-127 (qT/kT layout
    [dims(h0|h1), tokens]).
  - scores: per 128-key chunk, four concurrent 64x64 PE-array tiles
    (row group = head, col group = key half) compute both heads' scores
    at full-array MAC rate: st [128 keys, (h0 512q | h1 512q)] f32.
  - exp: one ACT activation per kc over both heads' scores [128, 1024];
    ACT is the bottleneck engine (~1.04us per kc, 128 kc total).
  - AV: four concurrent 64x64 tiles accumulate into acc_top/acc_bot
    [128,512] f32 (h0/h1 on complementary partitions, key-halves in
    separate banks to avoid row-group bank conflicts).
  - denominator: DVE accumulates dacc += et per kc; at window end an
    all-ones [128,128] stationary matmul partition-reduces dacc AND
    broadcasts the per-query exp-sum to all partitions; reciprocal +
    (acc_top+acc_bot)*recip normalize lands aoutT [dims, tokens] fp16,
    which is directly the proj lhsT (no transposes anywhere).
  - qkv generation and proj run as PE fillers in the ACT-bound gaps,
    using PSUM banks 6-7.

PSUM banks: 0-1 acc_top/acc_bot, 2-5 st ping/pong, 6-7 fillers.
"""

import numpy as np
from contextlib import ExitStack
from collections import deque

import concourse.bass as bass
import concourse.mybir as mybir
import concourse.tile as tile
from concourse import bacc
from concourse.bass import ts
from concourse.bass_utils import run_bass_kernel_spmd

P = 128
B = 2
N = 2048
C = 1024
H = 16
D = 64
T = B * N            # 4096 tokens
KO = C // P          # 8 contraction chunks of 128
NCORES = 8
HPC = H // NCORES    # 2 heads per core
TB = 512             # token block for qkv tiles / matmul free dim
QW = 512             # query window
NKC = N // P         # 16 key chunks per batch
NW = N // QW         # 4 windows per batch
SCALE = C ** -0.5    # 1/32 - reference scales by embed_dim**-0.5

F16 = mybir.dt.float16
F32 = mybir.dt.float32
AF = mybir.ActivationFunctionType
ALU = mybir.AluOpType


def build_program(n_iters: int = 1, hw_loop: int = 1):
    nc = bacc.Bacc("TRN2", target_bir_lowering=False, debug=False)

    xT = nc.dram_tensor("xT", [C, T], F16, kind="ExternalInput")
    wqkv = nc.dram_tensor("wqkv", [C, 3 * P], F16, kind="ExternalInput")
    wproj = nc.dram_tensor("wproj", [P, C], F16, kind="ExternalInput")
    y = nc.dram_tensor("y", [T, C], F16, kind="ExternalOutput")

    xT_r = xT.rearrange("(o p) t -> p o t", p=P)
    wqkv_r = wqkv.rearrange("(o p) c -> p o c", p=P)

    with tile.TileContext(nc) as tc, ExitStack() as ctx:
        const = ctx.enter_context(tc.tile_pool(name="const", bufs=1))
        big = ctx.enter_context(tc.tile_pool(name="big", bufs=1))
        etp = ctx.enter_context(tc.tile_pool(name="etp", bufs=3))
        dap = ctx.enter_context(tc.tile_pool(name="dap", bufs=2))
        rbp = ctx.enter_context(tc.tile_pool(name="rbp", bufs=2))
        tmpp = ctx.enter_context(tc.tile_pool(name="tmpp", bufs=2))
        yp = ctx.enter_context(tc.tile_pool(name="yp", bufs=4))
        accp = ctx.enter_context(tc.tile_pool(name="accp", bufs=1, space="PSUM"))
        stp = ctx.enter_context(tc.tile_pool(name="stp", bufs=2, space="PSUM"))
        fp = ctx.enter_context(tc.tile_pool(name="fp", bufs=2, space="PSUM"))

        ones_sb = const.tile([P, P], F16)
        nc.vector.memset(ones_sb[:], 1.0)
        wqkv_sb = const.tile([P, KO, 3 * P], F16)
        nc.sync.dma_start(wqkv_sb[:], wqkv_r)
        wproj_sb = const.tile([P, C], F16)
        nc.sync.dma_start(wproj_sb[:], wproj[:])

        def body():
            xT_sb = big.tile([P, KO, T], F16, tag="xT")
            for t in range(T // TB):
                nc.sync.dma_start(xT_sb[:, :, ts(t, TB)], xT_r[:, :, ts(t, TB)])

            qT_sb = big.tile([P, T], F16, tag="qT")
            kT_sb = big.tile([P, T], F16, tag="kT")
            # v: [key-within-chunk, chunk, dims(h0 64 | h1 64)]
            v_sb = big.tile([P, T // P, P], F16, tag="v")
            # unnormalized attention output, [dims(h0|h1), tokens]
            aoutT_sb = big.tile([P, T], F16, tag="aoutT")

            # ---- emission helpers ------------------------------------
            def emit_qk_tile(m, dst, t):
                # dst[:, t*TB:(t+1)*TB] = (wqkv col block m).T @ x
                ps = fp.tile([P, TB], F32, tag="f", name="ps_qk")
                for k in range(KO):
                    nc.tensor.matmul(
                        ps[:],
                        lhsT=wqkv_sb[:, k, ts(m, P)],
                        rhs=xT_sb[:, k, ts(t, TB)],
                        start=(k == 0),
                        stop=(k == KO - 1),
                    )
                nc.vector.tensor_copy(dst[:, ts(t, TB)], ps[:])

            def emit_v_tile(t):
                # v_sb[:, t, :] = x(tokens t*128..) @ Wv   [tokens, dims]
                ps = fp.tile([P, TB], F32, tag="f", name="ps_v")
                for k in range(KO):
                    nc.tensor.matmul(
                        ps[:, :P],
                        lhsT=xT_sb[:, k, ts(t, P)],
                        rhs=wqkv_sb[:, k, 2 * P : 3 * P],
                        start=(k == 0),
                        stop=(k == KO - 1),
                    )
                nc.vector.tensor_copy(v_sb[:, t, :], ps[:, :P])

            def emit_proj_chunk(t):
                # y[t*128.., :] = aoutT[:, chunk].T @ wproj
                for nb in range(C // TB):
                    ps = fp.tile([P, TB], F32, tag="f", name="ps_pr")
                    nc.tensor.matmul(
                        ps[:],
                        lhsT=aoutT_sb[:, ts(t, P)],
                        rhs=wproj_sb[:, ts(nb, TB)],
                        start=True,
                        stop=True,
                    )
                    yt = yp.tile([P, TB], F16, tag="y")
                    nc.vector.tensor_copy(yt[:], ps[:])
                    nc.sync.dma_start(y[ts(t, P), ts(nb, TB)], yt[:])

            # ---- filler machinery (emission order = dep order) -------
            fillers = deque()
            emitted = set()

            def pop_filler():
                while fillers:
                    key, fn = fillers.popleft()
                    if key in emitted:
                        continue
                    emitted.add(key)
                    fn()
                    return

            def ensure_filler(key):
                if key in emitted:
                    return
                for k2, fn in fillers:
                    if k2 == key:
                        emitted.add(key)
                        fn()
                        return

            # ---- QKV lead: minimum to start window (b=0, w=0) --------
            emit_qk_tile(1, kT_sb, 0)        # kT tokens 0:512 (kc 0-3)
            emit_qk_tile(0, qT_sb, 0)        # qT tokens 0:512 (w0 queries)
            for t in range(1, 4):
                fillers.append((("qk", 1, t), lambda t=t: emit_qk_tile(1, kT_sb, t)))
            for t in range(1, 4):
                fillers.append((("qk", 0, t), lambda t=t: emit_qk_tile(0, qT_sb, t)))
            for t in range(16):
                fillers.append((("v", t), lambda t=t: emit_v_tile(t)))
            for t in range(4, 8):
                fillers.append((("qk", 1, t), lambda t=t: emit_qk_tile(1, kT_sb, t)))
            for t in range(4, 8):
                fillers.append((("qk", 0, t), lambda t=t: emit_qk_tile(0, qT_sb, t)))
            for t in range(16, 32):
                fillers.append((("v", t), lambda t=t: emit_v_tile(t)))

            # ---- attention -------------------------------------------
            for b in range(B):
                for w in range(NW):
                    q0 = b * N + w * QW      # global token index of queries
                    ensure_filler(("qk", 0, (b * N + w * QW) // TB))
                    dacc = [None, None]
                    for kc in range(NKC):
                        ensure_filler(("qk", 1, (b * NKC + kc) // 4))
                        k0 = b * N + kc * P  # global token index of keys
                        st = stp.tile([P, 2 * QW], F32, tag="st")
                        # 4 concurrent 64x64 score tiles:
                        #   st[0:64, 0:512]    h0, keys 0:64   (tile 0,0)
                        #   st[64:128, 0:512]  h0, keys 64:128 (tile 0,64)
                        #   st[0:64, 512:]     h1, keys 0:64   (tile 64,0)
                        #   st[64:128, 512:]   h1, keys 64:128 (tile 64,64)
                        for h in range(HPC):
                            hs = h * D
                            for kh in range(2):
                                nc.tensor.matmul(
                                    st[kh * D : (kh + 1) * D,
                                       h * QW : (h + 1) * QW],
                                    lhsT=kT_sb[hs : hs + D,
                                               k0 + kh * D : k0 + (kh + 1) * D],
                                    rhs=qT_sb[hs : hs + D, q0 : q0 + QW],
                                    start=True,
                                    stop=True,
                                    tile_position=(hs, kh * D),
                                )
                        et = etp.tile([P, 2 * QW], F16, tag="et")
                        nc.scalar.activation(et[:], st[:], AF.Exp, scale=SCALE)

                        # 4 concurrent 64x64 AV tiles (accumulate over kc):
                        bkc = b * NKC + kc
                        ensure_filler(("v", bkc))
                        acc_top = accp.tile([P, QW], F32, tag="acct",
                                            name="acc_top") if kc == 0 else acc_top
                        acc_bot = accp.tile([P, QW], F32, tag="accb",
                                            name="acc_bot") if kc == 0 else acc_bot
                        for h in range(HPC):
                            hs = h * D
                            for kh in range(2):
                                acc = acc_top if kh == 0 else acc_bot
                                nc.tensor.matmul(
                                    acc[hs : hs + D, :],
                                    lhsT=v_sb[kh * D : (kh + 1) * D, bkc,
                                              hs : hs + D],
                                    rhs=et[kh * D : (kh + 1) * D,
                                           h * QW : (h + 1) * QW],
                                    start=(kc == 0),
                                    stop=(kc == NKC - 1),
                                    tile_position=(kh * D, hs),
                                )

                        # denominator accumulation on DVE (ping-pong)
                        if kc == 0:
                            d = dap.tile([P, 2 * QW], F16, tag="dacc",
                                         name="dacc0")
                            nc.vector.tensor_copy(d[:], et[:])
                            dacc = [d, dap.tile([P, 2 * QW], F16, tag="dacc",
                                                name="dacc1")]
                        else:
                            src, dst = dacc[(kc + 1) % 2], dacc[kc % 2]
                            nc.vector.scalar_tensor_tensor(
                                dst[:], et[:], 1.0, src[:],
                                op0=ALU.mult, op1=ALU.add,
                            )
                        pop_filler()

                    dfin = dacc[(NKC - 1) % 2]
                    # partition-reduce + broadcast the denominator:
                    # dbc_h[p, q] = sum_k dacc[k, q] for every p
                    rb = rbp.tile([P, QW], F32, tag="rb")
                    for h in range(HPC):
                        dbc = fp.tile([P, QW], F32, tag="f", name="dbc")
                        nc.tensor.matmul(
                            dbc[:],
                            lhsT=ones_sb[:],
                            rhs=dfin[:, h * QW : (h + 1) * QW],
                            start=True,
                            stop=True,
                        )
                        # rb[h0 dims] = 1/denom_h0 ; rb[h1 dims] = 1/denom_h1
                        hs = h * D
                        nc.vector.reciprocal(rb[hs : hs + D, :],
                                             dbc[hs : hs + D, :])

                    # normalize: aoutT[:, q0:q0+QW] = (acc_top+acc_bot)*rb
                    tmp = tmpp.tile([P, QW], F32, tag="tmp")
                    nc.vector.tensor_copy(tmp[:], acc_top[:])
                    s2 = tmpp.tile([P, QW], F32, tag="tmp")
                    nc.vector.scalar_tensor_tensor(
                        s2[:], acc_bot[:], 1.0, tmp[:],
                        op0=ALU.mult, op1=ALU.add,
                    )
                    nc.vector.scalar_tensor_tensor(
                        aoutT_sb[:, q0 : q0 + QW], s2[:], 1.0, rb[:],
                        op0=ALU.mult, op1=ALU.mult,
                    )
                    # proj for these tokens becomes filler work
                    last_win = (b == B - 1 and w == NW - 1)
                    for qs in range(QW // P):
                        t = (q0 // P) + qs
                        if last_win:
                            emit_proj_chunk(t)
                        else:
                            fillers.append(
                                (("proj", t), lambda t=t: emit_proj_chunk(t)))

            while fillers:
                pop_filler()

        if hw_loop > 1:
            with tc.For_i(0, hw_loop, 1):
                body()
        else:
            for _ in range(n_iters):
                body()

    nc.compile()
    return nc


_CACHE = {}


def _get_program(n_iters: int = 1):
    if n_iters not in _CACHE:
        _CACHE[n_iters] = build_program(n_iters)
    return _CACHE[n_iters]


def make_core_inputs(x, W_qkv):
    """Shared per-core host prep; returns (xT16, [wqkv_c for c in range(8)])."""
    xT16 = np.ascontiguousarray(
        x.reshape(T, C).astype(np.float16, copy=False).T
    )
    wq = []
    for c in range(NCORES):
        lo, hi = 2 * c * 64, (2 * c + 2) * 64
        wq.append(
            np.ascontiguousarray(
                np.concatenate(
                    [W_qkv[:, lo:hi], W_qkv[:, C + lo : C + hi],
                     W_qkv[:, 2 * C + lo : 2 * C + hi]],
                    axis=1,
                ).astype(np.float16)
            )
        )
    return xT16, wq


def kernel(x, W_qkv, W_proj, b_proj):
    x = np.asarray(x, dtype=np.float32)
    W_qkv = np.asarray(W_qkv, dtype=np.float32)
    W_proj = np.asarray(W_proj, dtype=np.float32)
    b_proj = np.asarray(b_proj, dtype=np.float32)

    nc = _get_program(1)
    xT16, wq = make_core_inputs(x, W_qkv)
    in_maps = []
    for c in range(NCORES):
        lo, hi = 2 * c * 64, (2 * c + 2) * 64
        in_maps.append(
            {
                "xT": xT16,
                "wqkv": wq[c],
                "wproj": np.ascontiguousarray(W_proj[lo:hi, :].astype(np.float16)),
            }
        )

    res = run_bass_kernel_spmd(nc, in_maps, list(range(NCORES)))
    acc = np.zeros((T, C), dtype=np.float32)
    for c in range(NCORES):
        acc += res.results[c]["y"].astype(np.float32)
    acc += b_proj[None, :]
    return acc.reshape(B, N, C)
